# revision 32
# baseline (speedup 1.0000x reference)
"""Trainium2 Bass kernel for nn_DiffeqExactTraceMLP.

Math (B=1024, D=128, DH=64, H=512):
  h = MADE_fwd(x) + MADE_rev(x)                       # hollow conditioner
  u[b,i] = [t, x[b,i], h[b,i,:]]                      # [B, D, DH+2]
  y   = MLP(u)        (tanh, tanh, linear->scalar)    # per-dim MLP
  jac = exact JVP of MLP wrt the x slot of u

v6: DIM-sharded + weight AllGather. Core c handles dims {c, c+8, ..,
c+120} for the FULL batch. MADE L0/L1 are replicated (full batch, cheap);
MADE L2 is dim-sharded so each core only receives its 1/8 of the (masked)
W2. With MADE hidden units sorted by degree, the per-dim active rows of
W2 form a prefix (fwd net) / suffix-as-prefix (rev net); per psum piece
exactly 5 of 8 row-chunks are live, with a core-INDEPENDENT schedule
(m1cnt=[1,1,2,2,3,3,4,4], m2cnt reversed) -> uniform SPMD program.
W1 (masked) becomes block-upper-triangular after sorting: 10 of 16 blocks.

The replicated weights (xT, W0s, W1s packed, dimwise W1/W0x) are sharded
1/8 per core inside wA and reassembled on device with two AllGathers
(round 1 = MADE weights, gating compute start; round 2 = dimwise weights,
overlapping MADE). Each piece's <=64-row boundary W2 block is shipped
half-height (two pieces share one 128-col slot). Per-core input: wA
[128, 5658] bf16 (~1.45 MB); f32 scalars carried as bf16 hi+lo pairs
(the replicated 40 columns ride in gather round 1, only the per-core b2
columns ship direct). Output yj [2, 16384] bf16, rows r = iloc*1024 + b.

Device schedule per core:
  AllGather shards -> repl_sb [128, 7248]
  h1 = relu(W0s^T xT + b0s); h2 = relu(W1s^T h1 + b1s)   (both nets,
      feature-on-partition, full batch, triangular W1 blocks)
  per piece g (2 local dims): psum[128=(j,k), 1024b] = 4 full + 1 short
      packed W2 chunk matmuls; +b2 bias on the psum->SBUF copy; 2
      partition-shift DMAs scatter into UT (row 0 = x slot via DMA).
  dimwise MLP over 16 row blocks of 1024:
    z1  = tanh(W0x^T UT + bias0_eff), z1d = a - a*z1^2
    p2  = W1^T z1, q2n = W1^T z1d     (same stationary weights)
    z2  = tanh(p2 + d_b1); Sy += w2[m]*z2
    Sj += (z2^2-1) * (q2n * -w2[m])   (per-partition DVE scale)
    y/jac rows accumulate into two persistent [32, 512] psum tiles via
    staircase-selector matmuls; 2 copies + 2 DMAs write yj.
Matmuls in bf16 (fp32 PSUM accumulation).
"""

import numpy as np
import ml_dtypes

B, D, DH, H = 1024, 128, 64, 512
NCORES = 8
DPC = D // NCORES         # dims per core = 16
ROWS = DPC * B            # dimwise rows per core = 16384
RBLK = 1024               # dimwise row-block
NBLK = ROWS // RBLK       # 16
KIN = DH + 1              # 65 = [x, h0..h63]

M1CNT = [1, 1, 2, 2, 3, 3, 4, 4]   # live fwd-net W2 row-chunks per piece
BI = [0, 1, 3, 6]                  # packed W1 block start per out-chunk m

_BF = ml_dtypes.bfloat16

# replicated blob [128, RCOLS] bf16 (sharded 1/8 per core, AllGather'd in
# two rounds: cols 0:RC1 first -- MADE needs them early -- then the rest)
R_XT = 0          # [128, 1024] x^T (dims on partitions)
R_W0 = {"m1": 1024, "m2": 1536}    # [128, 512] masked+sorted W0
R_W1 = {"m1": 2048, "m2": 3328}    # [128, 1280] 10 packed triangular blocks
R_WDH = 4608     # [128, 40] replicated f32 scalars, bf16 hi part
R_WDL = 4648     # lo part
RC1 = 4688
R_DW1 = 4688      # [128, 2048] dimwise W1, 4 row-chunks
R_DW0X = 6736     # [65, 512] dimwise W0 rows [x, h0..h63]
RCOLS = 7248
SHC1 = RC1 // NCORES           # 576
SHC2 = (RCOLS - RC1) // NCORES  # 320

# wA [128, A_COLS] bf16 column offsets
A_SH1 = 0         # [128, 586] this core's shard of gather round 1
A_SH2 = 586       # [128, 320] round 2
A_XU = 906        # [128, 128] per-core x-slot (flattens to UT row 0)
# W2 pack: per piece g, 4 full blocks at A_W2F+g*512; the per-piece short
# (<=64-row boundary) block of pieces (2u, 2u+1) shares slot u of A_W2S
# (partitions 0:64 / 64:128).
A_W2F = 1034      # [128, 4096]
A_W2S = 5130      # [128, 512]
A_WDHI = 5642     # [128, 8] per-core b2 scalars, bf16 hi part
A_WDLO = 5650     # lo part
A_COLS = 5658

# per piece g: the 4 full (net, chunk) blocks and the short one
W2SCHED = []
for _g in range(8):
    _n1, _n2 = M1CNT[_g], 5 - M1CNT[_g]
    if _g % 2 == 0:
        fulls = [("m1", _c) for _c in range(_n1 - 1)] + \
                [("m2", _c) for _c in range(_n2)]
        short = ("m1", _n1 - 1)
    else:
        fulls = [("m1", _c) for _c in range(_n1)] + \
                [("m2", _c) for _c in range(_n2 - 1)]
        short = ("m2", _n2 - 1)
    W2SCHED.append((fulls, short))
# wd (f32, [128, 48]) column offsets (chunk-col layout: v[c*128+p] at [p, c])
D_B0C = {"m1": 0, "m2": 4}
D_B1C = {"m1": 8, "m2": 12}
D_BIAS0, D_B1D, D_AC, D_NEGAC, D_W2, D_NW2, D_B2C = 16, 20, 24, 28, 32, 36, 40
D_COLS = 48


def _bf(x):
    return np.ascontiguousarray(np.asarray(x, dtype=np.float32).astype(_BF))


def _chunk_col(v):
    """[512] -> [128, 4] with v[c*128 + p] at [p, c] (per-partition scalars)."""
    return np.ascontiguousarray(np.asarray(v, np.float32).reshape(4, 128).T)


_NC_CACHE = {}


def _build_nc():
    import os
    unroll = int(os.environ.get("BENCH_UNROLL", "1"))
    # BENCH_MODE: "full" (graded; gather+compute per rep), "gather" (only the
    # gather chain repeats; one compute pass at the end), "compute" (gather
    # once; compute repeats). Only affects local benchmarking.
    mode = os.environ.get("BENCH_MODE", "full")
    key = (unroll, mode)
    if key in _NC_CACHE:
        return _NC_CACHE[key]
    import concourse.bacc as bacc
    import concourse.mybir as mybir
    from concourse.tile import TileContext

    dt = mybir.dt
    AF = mybir.ActivationFunctionType
    OP = mybir.AluOpType

    nc = bacc.Bacc(None, target_bir_lowering=False, num_devices=NCORES)

    wA = nc.declare_dram_parameter("wA", [128, A_COLS], dt.bfloat16, isOutput=False)
    yj = nc.declare_dram_parameter("yj", [2, ROWS], dt.bfloat16, isOutput=True)

    with TileContext(nc) as tc:
        _pools = []

        def _pool(**kw):
            p = tc.alloc_tile_pool(**kw)
            _pools.append(p)
            return p

        cpool = _pool(name="const", bufs=1)
        h1pool = _pool(name="made_h1", bufs=4)
        h2pool = _pool(name="made_h2", bufs=8)
        mgpool = _pool(name="made_g", bufs=3)
        zpool = _pool(name="z", bufs=8)
        z1dpool = _pool(name="z1d", bufs=8)
        z2pool = _pool(name="z2", bufs=3)
        tpool = _pool(name="tmp", bufs=3)
        sqpool = _pool(name="sq", bufs=3)
        spool = _pool(name="S", bufs=3)
        dram = _pool(name="dram", bufs=1, space="DRAM")

        _dma_engines = [nc.sync, nc.scalar]
        _dma_i = [0]

        def dma_rr(out, in_):
            eng = _dma_engines[_dma_i[0] % len(_dma_engines)]
            _dma_i[0] += 1
            eng.dma_start(out=out, in_=in_)

        def ctile(shape, dtype, name):
            return cpool.tile(list(shape), dtype, tag=name, name=name)

        # repeated body for benchmarking (BENCH_UNROLL>1); rep results identical
        def emit_gather_consts():
            # ---- replicated-weight AllGather, 2 rounds (round 1 gates MADE,
            # round 2 only gates the dimwise phase and overlaps MADE) ----
            rp = ctile((128, RCOLS), dt.bfloat16, "repl_sb")
            ag = []
            for i, (a0, shc, r0) in enumerate(
                    ((A_SH1, SHC1, 0), (A_SH2, SHC2, RC1))):
                agin = dram.tile([128, shc], dt.bfloat16, tag=f"agin{i}",
                                 name=f"agin{i}")
                agout = dram.tile([128 * NCORES, shc], dt.bfloat16,
                                  tag=f"agout{i}", name=f"agout{i}",
                                  addr_space="Shared")
                nc.gpsimd.dma_start(out=agin[:], in_=wA[:, a0:a0 + shc])
                ag.append((agin, agout, shc, r0))

            # ---- per-core direct loads BEFORE the collective-dependent
            # unpack DMAs: they share the sync/scalar queues, and anything
            # emitted after the unpacks would be head-of-line blocked behind
            # the gather even with no data dependency on it. ----
            wd_hi = ctile((128, 8), dt.bfloat16, "wd_hi")
            nc.sync.dma_start(out=wd_hi[:], in_=wA[:, A_WDHI:A_WDHI + 8])
            wd_lo = ctile((128, 8), dt.bfloat16, "wd_lo")
            nc.sync.dma_start(out=wd_lo[:], in_=wA[:, A_WDLO:A_WDLO + 8])
            wd_sb = ctile((128, D_COLS), dt.float32, "wd_sb")
            nc.vector.tensor_tensor(wd_sb[:, 40:48], wd_hi[:], wd_lo[:], op=OP.add)
            # w2p_sb per piece: 4 full blocks (cols g*640..+512) + 1 short
            # 64-row block (cols g*640+512..+640, partitions 0:64)
            w2p_sb = ctile((128, 5120), dt.bfloat16, "w2p_sb")
            for g in range(8):
                nc.scalar.dma_start(
                    out=w2p_sb[:, g * 640:g * 640 + 512],
                    in_=wA[:, A_W2F + g * 512:A_W2F + (g + 1) * 512])
                nc.scalar.dma_start(
                    out=w2p_sb[0:64, g * 640 + 512:g * 640 + 640],
                    in_=wA[64 * (g % 2):64 * (g % 2) + 64,
                           A_W2S + (g // 2) * 128:A_W2S + (g // 2 + 1) * 128])
            zsel = ctile((128, 63), dt.bfloat16, "zsel")
            nc.vector.memset(zsel[:], 0.0)
            nc.vector.memset(zsel[:, 31:32], 1.0)

            # Each round's unpack DMAs are emitted right after its collective
            # (round 1 unpacks must not queue behind collective 2), and on the
            # gpsimd DMA queue: these DMAs stall on the collectives, and on
            # sync/scalar they would head-of-line block the UT-assembly DMAs
            # of the compute phase behind gather round 2.
            for agin, agout, shc, r0 in ag:
                nc.gpsimd.collective_compute(
                    "AllGather",
                    mybir.AluOpType.bypass,
                    replica_groups=[list(range(NCORES))],
                    ins=[agin.opt()],
                    outs=[agout.opt()],
                )
                for r in range(NCORES):
                    nc.gpsimd.dma_start(
                        out=rp[:, r0 + r * shc:r0 + (r + 1) * shc],
                        in_=agout[128 * r:128 * (r + 1), :])
            nc.vector.tensor_tensor(wd_sb[:, 0:40], rp[:, R_WDH:R_WDH + 40],
                                    rp[:, R_WDL:R_WDL + 40], op=OP.add)
            return rp, w2p_sb, wd_sb, zsel

        def emit_compute(rp, w2p_sb, wd_sb, zsel):
            def wdc(col):
                return wd_sb[:, col:col + 1]

            UT = ctile((KIN, ROWS), dt.bfloat16, "UT")
            outsb_y = ctile((32, 512), dt.bfloat16, "outsb_y")
            outsb_j = ctile((32, 512), dt.bfloat16, "outsb_j")

            # ---------------- MADE (both orderings, full batch) ----------------
            madeps = tc.alloc_tile_pool(name="madeps", bufs=3, space="PSUM")
            h2 = {}
            for p in ("m1", "m2"):
                h1 = {}
                for m in range(4):
                    ps = madeps.tile([128, B], dt.float32, tag="mps", name="mps")
                    for s in range(2):
                        sl = slice(s * 512, (s + 1) * 512)
                        nc.tensor.matmul(
                            ps[:, sl],
                            rp[:, R_W0[p] + m * 128:R_W0[p] + (m + 1) * 128],
                            rp[:, R_XT + s * 512:R_XT + (s + 1) * 512],
                            start=True, stop=True,
                        )
                    h = h1pool.tile([128, B], dt.bfloat16, tag="h1", name="h1")
                    nc.scalar.activation(h[:], ps[:], AF.Relu, bias=wdc(D_B0C[p] + m))
                    h1[m] = h
                for m in range(4):
                    ps = madeps.tile([128, B], dt.float32, tag="mps", name="mps")
                    for s in range(2):
                        sl = slice(s * 512, (s + 1) * 512)
                        for k in range(m + 1):
                            o = R_W1[p] + (BI[m] + k) * 128
                            nc.tensor.matmul(
                                ps[:, sl], rp[:, o:o + 128], h1[k][:, sl],
                                start=(k == 0), stop=(k == m),
                            )
                    h = h2pool.tile([128, B], dt.bfloat16, tag="h2", name="h2")
                    nc.scalar.activation(h[:], ps[:], AF.Relu, bias=wdc(D_B1C[p] + m))
                    h2[p, m] = h

            # x slot -> UT row 0
            nc.sync.dma_start(out=UT[0:1, :], in_=wA[:, A_XU:A_XU + 128])

            # MADE L2: per piece g, 4 full + 1 short chunk -> psum[(j,k), 1024b]
            for g in range(8):
                fulls, (snet, sch) = W2SCHED[g]
                ps = madeps.tile([128, B], dt.float32, tag="mps", name="mps")
                for s in range(2):
                    sl = slice(s * 512, (s + 1) * 512)
                    for j, (net, ch) in enumerate(fulls):
                        nc.tensor.matmul(
                            ps[:, sl],
                            w2p_sb[:, g * 640 + j * 128:g * 640 + (j + 1) * 128],
                            h2[net, ch][:, sl],
                            start=(j == 0), stop=False,
                        )
                    nc.tensor.matmul(
                        ps[:, sl],
                        w2p_sb[0:64, g * 640 + 512:g * 640 + 640],
                        h2[snet, sch][0:64, sl],
                        start=False, stop=True,
                    )
                mg = mgpool.tile([128, B], dt.bfloat16, tag="mg", name="mg")
                nc.scalar.activation(mg[:, 0:512], ps[:, 0:512], AF.Identity,
                                     bias=wdc(D_B2C + g))
                nc.vector.tensor_scalar(mg[:, 512:1024], ps[:, 512:1024],
                                        wdc(D_B2C + g), None, op0=OP.add)
                dma_rr(UT[1:65, (2 * g) * B:(2 * g) * B + B], mg[0:64, :])
                dma_rr(UT[1:65, (2 * g + 1) * B:(2 * g + 1) * B + B], mg[64:128, :])

            madeps.release()
            pspool = tc.alloc_tile_pool(name="ps", bufs=3, space="PSUM")
            psfp = tc.alloc_tile_pool(name="psf", bufs=1, space="PSUM")
            psy = psfp.tile([32, 512], dt.float32, tag="psy", name="psy")
            psj = psfp.tile([32, 512], dt.float32, tag="psj", name="psj")

            # ---------------- dimwise MLP over row blocks ----------------
            # L0 for block b+1 is emitted between L1(b) and final(b): PE fills
            # the stall where it would wait on ACT/DVE producing z2/Sy/Sj(b),
            # and z1(b+1) is ready before L1(b+1) begins.
            def do_L0(b):
                base = b * RBLK
                z1 = {}
                z1d = {}
                for m in range(4):
                    zt = zpool.tile([128, RBLK], dt.bfloat16, tag="z1", name="z1")
                    ps = pspool.tile([128, RBLK], dt.float32, tag="ps", name="psL0")
                    for s in range(2):
                        nc.tensor.matmul(
                            ps[:, s * 512:(s + 1) * 512],
                            rp[0:KIN, R_DW0X + m * 128:R_DW0X + (m + 1) * 128],
                            UT[:, base + s * 512: base + (s + 1) * 512],
                            start=True, stop=True,
                        )
                    nc.scalar.activation(zt[:], ps[:], AF.Tanh, bias=wdc(D_BIAS0 + m))
                    sq = sqpool.tile([128, RBLK], dt.bfloat16, tag="sq1", name="sq1")
                    nc.vector.tensor_tensor(sq[:], zt[:], zt[:], op=OP.mult)
                    zd = z1dpool.tile([128, RBLK], dt.bfloat16, tag="z1d", name="z1d")
                    nc.vector.tensor_scalar(
                        zd[:], sq[:], wdc(D_NEGAC + m), wdc(D_AC + m),
                        op0=OP.mult, op1=OP.add,
                    )
                    z1[m] = zt
                    z1d[m] = zd
                return z1, z1d

            cur = do_L0(0)
            for b in range(NBLK):
                z1, z1d = cur
                Sy = spool.tile([128, RBLK], dt.bfloat16, tag="Sy", name="Sy")
                Sj = spool.tile([128, RBLK], dt.bfloat16, tag="Sj", name="Sj")
                for m in range(4):
                    p2 = pspool.tile([128, RBLK], dt.float32, tag="ps", name="p2ps")
                    q2n = pspool.tile([128, RBLK], dt.float32, tag="ps", name="q2nps")
                    for k in range(4):
                        lhs = rp[:, R_DW1 + k * H + m * 128:R_DW1 + k * H + (m + 1) * 128]
                        for s in range(2):
                            sl = slice(s * 512, (s + 1) * 512)
                            nc.tensor.matmul(
                                p2[:, sl], lhs, z1[k][:, sl],
                                start=(k == 0), stop=(k == 3),
                            )
                            nc.tensor.matmul(
                                q2n[:, sl], lhs, z1d[k][:, sl],
                                start=(k == 0), stop=(k == 3),
                            )
                    z2t = z2pool.tile([128, RBLK], dt.bfloat16, tag="z2", name="z2")
                    nc.scalar.activation(z2t[:], p2[:], AF.Tanh, bias=wdc(D_B1D + m))
                    sq = sqpool.tile([128, RBLK], dt.bfloat16, tag="sq2", name="sq2")
                    nc.vector.tensor_tensor(sq[:], z2t[:], z2t[:], op=OP.mult)
                    # Sj += (z2^2 - 1) * (q2n * -w2[m])  ( = w2 * z2d )
                    p2nw = tpool.tile([128, RBLK], dt.bfloat16, tag="p2nw", name="p2nw")
                    nc.vector.tensor_scalar(
                        p2nw[:], q2n[:], wdc(D_NW2 + m), None, op0=OP.mult,
                    )
                    if m == 0:
                        nc.vector.scalar_tensor_tensor(
                            Sj[:], sq[:], 1.0, p2nw[:], op0=OP.subtract, op1=OP.mult,
                        )
                        nc.vector.tensor_scalar(
                            Sy[:], z2t[:], wdc(D_W2 + 0), None, op0=OP.mult,
                        )
                    else:
                        zdt = tpool.tile([128, RBLK], dt.bfloat16, tag="zdt", name="zdt")
                        nc.vector.scalar_tensor_tensor(
                            zdt[:], sq[:], 1.0, p2nw[:], op0=OP.subtract, op1=OP.mult,
                        )
                        nc.vector.tensor_tensor(Sj[:], zdt[:], Sj[:], op=OP.add)
                        nc.vector.scalar_tensor_tensor(
                            Sy[:], z2t[:], wdc(D_W2 + m), Sy[:],
                            op0=OP.mult, op1=OP.add,
                        )
                if b + 1 < NBLK:
                    cur = do_L0(b + 1)
                for s in range(2):
                    sl = slice(s * 512, (s + 1) * 512)
                    jt = 2 * b + s
                    sel = zsel[:, 31 - jt:63 - jt]
                    nc.tensor.matmul(psy[:], sel, Sy[:, sl],
                                     start=(jt == 0), stop=(jt == 31))
                    nc.tensor.matmul(psj[:], sel, Sj[:, sl],
                                     start=(jt == 0), stop=(jt == 31))

            nc.scalar.activation(outsb_y[:], psy[:], AF.Copy)
            nc.vector.tensor_copy(outsb_j[:], psj[:])
            nc.sync.dma_start(out=yj[0:1, :], in_=outsb_y[:])
            nc.sync.dma_start(out=yj[1:2, :], in_=outsb_j[:])

            psfp.release()
            pspool.release()

        if mode == "compute":
            C = emit_gather_consts()
            for _rep in range(unroll):
                emit_compute(*C)
        elif mode == "gather":
            for _rep in range(unroll):
                C = emit_gather_consts()
            emit_compute(*C)
        else:
            for _rep in range(unroll):
                emit_compute(*emit_gather_consts())
        for p in reversed(_pools):
            p.release()

    nc.compile()
    _NC_CACHE[key] = nc
    return nc


def _host_prep(inputs):
    """Build the per-core input maps (numpy only)."""
    t = np.asarray(inputs["t"], np.float32)
    x = np.asarray(inputs["x"], np.float32)

    deg_h = np.arange(H) % (D - 1)
    sig = np.argsort(deg_h, kind="stable")
    deg_in = {"m1": np.arange(D), "m2": np.arange(D)[::-1].copy()}

    d_W0 = np.asarray(inputs["d_W0"], np.float32)
    d_b0 = np.asarray(inputs["d_b0"], np.float32)
    d_W1 = np.asarray(inputs["d_W1"], np.float32)
    w2 = np.asarray(inputs["d_W2"], np.float32)[:, 0]

    W0s, W1s, W2s, b0s, b1s = {}, {}, {}, {}, {}
    Mh = (deg_h[:, None] <= deg_h[None, :]).astype(np.float32)
    for p in ("m1", "m2"):
        M0 = (deg_in[p][:, None] <= deg_h[None, :]).astype(np.float32)
        Mo = (deg_h[:, None] < deg_in[p][None, :]).astype(np.float32)
        W0s[p] = (np.asarray(inputs[p + "_W0"], np.float32) * M0)[:, sig]
        W1s[p] = (np.asarray(inputs[p + "_W1"], np.float32) * Mh)[sig][:, sig]
        W2s[p] = (np.asarray(inputs[p + "_W2"], np.float32)
                  * np.tile(Mo, (1, DH)))[sig, :]
        b0s[p] = np.asarray(inputs[p + "_b0"], np.float32)[sig]
        b1s[p] = np.asarray(inputs[p + "_b1"], np.float32)[sig]
    b2s = np.asarray(inputs["m1_b2"], np.float32) + np.asarray(inputs["m2_b2"], np.float32)

    # replicated blob
    blob = np.zeros((128, RCOLS), np.float32)
    blob[:, R_XT:R_XT + B] = x.T
    for p in ("m1", "m2"):
        blob[:, R_W0[p]:R_W0[p] + H] = W0s[p]
        for m in range(4):
            for k in range(m + 1):
                o = R_W1[p] + (BI[m] + k) * 128
                blob[:, o:o + 128] = \
                    W1s[p][128 * k:128 * k + 128, 128 * m:128 * m + 128]
    for k in range(4):
        blob[:, R_DW1 + k * 512:R_DW1 + (k + 1) * 512] = d_W1[128 * k:128 * k + 128, :]
    blob[0:KIN, R_DW0X:R_DW0X + H] = d_W0[1:, :]

    wDm = np.zeros((128, D_COLS), np.float32)
    for p in ("m1", "m2"):
        wDm[:, D_B0C[p]:D_B0C[p] + 4] = _chunk_col(b0s[p])
        wDm[:, D_B1C[p]:D_B1C[p] + 4] = _chunk_col(b1s[p])
    wDm[:, D_BIAS0:D_BIAS0 + 4] = _chunk_col(d_b0 + t[0] * d_W0[0, :])
    wDm[:, D_B1D:D_B1D + 4] = _chunk_col(np.asarray(inputs["d_b1"], np.float32))
    a = d_W0[1, :]
    wDm[:, D_AC:D_AC + 4] = _chunk_col(a)
    wDm[:, D_NEGAC:D_NEGAC + 4] = _chunk_col(-a)
    wDm[:, D_W2:D_W2 + 4] = _chunk_col(w2)
    wDm[:, D_NW2:D_NW2 + 4] = _chunk_col(-w2)
    wc_hi = wDm[:, 0:40].astype(_BF)
    wc_lo = (wDm[:, 0:40] - wc_hi.astype(np.float32)).astype(_BF)
    blob[:, R_WDH:R_WDH + 40] = wc_hi.astype(np.float32)
    blob[:, R_WDL:R_WDL + 40] = wc_lo.astype(np.float32)
    blob_bf = blob.astype(_BF)

    # per-piece partition p -> (global dim, k) for core c:
    #   d = 8*(2g + p//64) + c, k = p%64
    pp = np.arange(128)
    kk = pp % 64
    jj = pp // 64

    in_maps = []
    for c in range(NCORES):
        wAc = np.zeros((128, A_COLS), np.float32)
        wAc[:, A_SH1:A_SH1 + SHC1] = \
            blob_bf[:, c * SHC1:(c + 1) * SHC1].astype(np.float32)
        wAc[:, A_SH2:A_SH2 + SHC2] = \
            blob_bf[:, RC1 + c * SHC2:RC1 + (c + 1) * SHC2].astype(np.float32)
        # x slot: region[p, cb] = x[(p%8)*128 + cb, 8*(p//8) + c]
        prow = np.arange(128)
        wAc[:, A_XU:A_XU + 128] = x[(prow[:, None] % 8) * 128 + np.arange(128)[None, :],
                                    8 * (prow[:, None] // 8) + c]
        # packed W2: block (net, ch) cols p -> W2s[net][chunk row, k*128+d];
        # 4 fulls per piece + the short boundary block (rows 0:64, paired
        # into A_W2S slot g//2 at partition half g%2)
        for g in range(8):
            dcol = 8 * (2 * g + jj) + c
            colidx = kk * 128 + dcol
            fulls, (snet, sch) = W2SCHED[g]
            for j, (net, ch) in enumerate(fulls):
                off = A_W2F + g * 512 + j * 128
                wAc[:, off:off + 128] = W2s[net][128 * ch:128 * ch + 128, colidx]
            sblk = W2s[snet][128 * sch:128 * sch + 64, colidx]
            assert np.all(W2s[snet][128 * sch + 64:128 * (sch + 1), colidx] == 0)
            off = A_W2S + (g // 2) * 128
            wAc[64 * (g % 2):64 * (g % 2) + 64, off:off + 128] = sblk
        b2c = np.zeros((128, 8), np.float32)
        for g in range(8):
            b2c[:, g] = b2s[kk * 128 + 8 * (2 * g + jj) + c]
        b2_hi = b2c.astype(_BF)
        b2_lo = (b2c - b2_hi.astype(np.float32)).astype(_BF)
        wAc[:, A_WDHI:A_WDHI + 8] = b2_hi.astype(np.float32)
        wAc[:, A_WDLO:A_WDLO + 8] = b2_lo.astype(np.float32)
        in_maps.append({"wA": wAc.astype(_BF)})
    return in_maps


def kernel(**inputs):
    from concourse.bass_utils import run_bass_kernel_spmd

    nc = _build_nc()
    in_maps = _host_prep(inputs)
    res = run_bass_kernel_spmd(nc, in_maps, list(range(NCORES)))

    d_b2 = np.asarray(inputs["d_b2"], np.float32)
    y = np.empty((B, D), np.float32)
    jac = np.empty((B, D), np.float32)
    for c in range(NCORES):
        yjc = np.asarray(res.results[c]["yj"], np.float32)
        y[:, c::8] = yjc[0].reshape(DPC, B).T
        jac[:, c::8] = yjc[1].reshape(DPC, B).T
    y += d_b2[0]
    return np.asarray(y, np.float32), np.asarray(jac, np.float32)
